# revision 1
# baseline (speedup 1.0000x reference)
"""Neighbour3dAttnProcessor Trainium2 kernel.

3D neighborhood attention (NATTEN, window 7x7x7 over T=16,H=32,W=32, 8 heads,
hd=64) + QKV/output projections, sharded over 8 NeuronCores by the H axis:
core i owns query rows h in [4i, 4i+4) and receives a 10-row K/V halo slice
(zero-padded at the borders; padding is excluded by the attention mask).

Attention-mask trick: scores are computed as K_aug^T @ Q_aug where rows 64..121
of the contraction carry one-hot key-position features (value -30720) on the
K side and {0,1} window-violation indicators on the Q side - the matmul then
produces raw_score - 30720 * (#violated window axes), so exp() underflows to
exactly 0 for every non-neighbor pair.  No separate masking pass is needed.

Tokens are reordered host-side to (w, h, t) so each query block (a pair of
w-columns, nq=128) attends a single contiguous run of 1280 key tokens = ten
128-key chunks.  Scores are produced keys-on-partitions so the AV stage needs
no transposes; a ones-column appended to V yields softmax denominators, and
normalization is a reciprocal + rank-1 broadcast matmul applied before the
output projection.
"""

import numpy as np
import ml_dtypes

import concourse.bass as bass
import concourse.tile as tile
from concourse import bacc, mybir
from concourse.bass_utils import run_bass_kernel_spmd

BF16 = mybir.dt.bfloat16
F32 = mybir.dt.float32
F32R = mybir.dt.float32r

T, H, W = 16, 32, 32
KT = KS = 7
HEADS, HD = 8, 64
C = HEADS * HD  # 512

N_CORES = 8
RH = 4          # own h rows per core
KVH = 10        # halo h rows per core
NQ = W * RH * T       # 2048 query tokens per core, order (w, j, t)
NKV = W * KVH * T     # 5120 kv tokens per core, order (w, hl, t)
WCOL = KVH * T        # 160 kv tokens per w column

R_T, R_H, R_W = 16, 10, 32
NAUG = R_T + R_H + R_W          # 58
KDIM = HD + NAUG                # 122 used contraction rows
KPAD = 128                      # padded to 128 (zeros) -> FWL eligible
NEG = -30720.0

NBLK = NQ // 128                # 16 query blocks (one per w-column pair)
NCHUNK = NKV // 128             # 40 key chunks


def _block_chunks(b):
    """Key chunk index range for query block b (w columns 2b, 2b+1)."""
    ws = min(max(2 * b - 3, 0), W - 8)
    lo = ws * WCOL
    hi = lo + 8 * WCOL
    return lo // 128, -(-hi // 128)


def build_nc():
    nc = bacc.Bacc(None, target_bir_lowering=False)

    xT = nc.declare_dram_parameter("xT", [C, NKV], BF16, isOutput=False)
    xTq = nc.declare_dram_parameter("xTq", [C, NQ], BF16, isOutput=False)
    wq = nc.declare_dram_parameter("wq", [C, C], BF16, isOutput=False)
    wk = nc.declare_dram_parameter("wk", [C, C], BF16, isOutput=False)
    wv = nc.declare_dram_parameter("wv", [C, C], BF16, isOutput=False)
    wo = nc.declare_dram_parameter("wo", [C, C], BF16, isOutput=False)
    Fm = nc.declare_dram_parameter("Fm", [KPAD - HD, NKV], BF16, isOutput=False)
    Gm = nc.declare_dram_parameter("Gm", [KPAD - HD, NQ], BF16, isOutput=False)
    one64 = nc.declare_dram_parameter("one64", [1, HD], F32R, isOutput=False)
    out = nc.declare_dram_parameter("out", [NQ, C], F32, isOutput=True)

    with tile.TileContext(nc) as tc:
        with (
            tc.tile_pool(name="persist", bufs=1) as pp,
            tc.tile_pool(name="stream", bufs=2) as sp,
            tc.tile_pool(name="psum", bufs=2, space="PSUM") as qq,
        ):
            # ---- persistent sbuf tiles ----
            ka = [pp.tile([KPAD, NKV], BF16, name=f"ka{h}", tag=f"ka{h}")
                  for h in range(HEADS)]
            qa = [pp.tile([KPAD, NQ], BF16, name=f"qa{h}", tag=f"qa{h}")
                  for h in range(HEADS)]
            vt = [pp.tile([128, 8 * 65], BF16, name=f"vt{c}", tag=f"vt{c}")
                  for c in range(NCHUNK)]
            ot = [pp.tile([128, NQ], BF16, name=f"ot{p}", tag=f"ot{p}")
                  for p in range(4)]
            wos = [pp.tile([128, C], BF16, name=f"wos{k}", tag=f"wos{k}")
                   for k in range(4)]
            ones = pp.tile([1, HD], F32R, name="ones", tag="ones")

            # ---- constant loads / initialization ----
            for k in range(4):
                nc.sync.dma_start(out=wos[k], in_=wo[128 * k:128 * (k + 1), :])
            for h in range(HEADS):
                # F/G arrive zero-padded to 64 rows, so [64:128) is fully
                # written by one DMA each.
                nc.sync.dma_start(out=ka[h][HD:KPAD, :], in_=Fm[:, :])
                nc.sync.dma_start(out=qa[h][HD:KPAD, :], in_=Gm[:, :])
            nc.sync.dma_start(out=ones[:, :], in_=one64[:, :])
            for c in range(NCHUNK):
                # fill with 1.0; the V-projection copy then overwrites the 64
                # value columns per head, leaving each head's 65th column at 1
                # (the softmax-denominator ones column).
                nc.gpsimd.memset(vt[c][:, :], 1.0)

            # ---- phase 1: QKV projections ----
            with tc.tile_pool(name="w1", bufs=1) as wp:
                wqs = [wp.tile([128, C], BF16, name=f"wqs{k}", tag=f"wqs{k}")
                       for k in range(4)]
                wks = [wp.tile([128, C], BF16, name=f"wks{k}", tag=f"wks{k}")
                       for k in range(4)]
                wvs = [wp.tile([128, C], BF16, name=f"wvs{k}", tag=f"wvs{k}")
                       for k in range(4)]
                for k in range(4):
                    nc.sync.dma_start(out=wqs[k], in_=wq[128 * k:128 * (k + 1), :])
                    nc.sync.dma_start(out=wks[k], in_=wk[128 * k:128 * (k + 1), :])
                    nc.sync.dma_start(out=wvs[k], in_=wv[128 * k:128 * (k + 1), :])

                # K and V over all NKV tokens
                for n in range(NKV // 512):
                    xs = [wp.tile([128, 512], BF16, name=f"xs{n}_{k}",
                                  tag=f"xs{k}", bufs=2) for k in range(4)]
                    for k in range(4):
                        nc.sync.dma_start(
                            out=xs[k],
                            in_=xT[128 * k:128 * (k + 1), 512 * n:512 * (n + 1)])
                    for p in range(4):  # head pairs
                        ps = qq.tile([128, 512], F32, name=f"pk{n}_{p}", tag="mm")
                        for k in range(4):
                            nc.tensor.matmul(ps[:, :],
                                             wks[k][:, 128 * p:128 * (p + 1)],
                                             xs[k][:, :],
                                             start=(k == 0), stop=(k == 3))
                        if p % 2 == 0:
                            nc.scalar.copy(ka[2 * p][0:HD, 512 * n:512 * (n + 1)],
                                           ps[0:HD, :])
                            nc.scalar.copy(
                                ka[2 * p + 1][0:HD, 512 * n:512 * (n + 1)],
                                ps[HD:128, :])
                        else:
                            nc.vector.tensor_copy(
                                ka[2 * p][0:HD, 512 * n:512 * (n + 1)],
                                ps[0:HD, :])
                            nc.vector.tensor_copy(
                                ka[2 * p + 1][0:HD, 512 * n:512 * (n + 1)],
                                ps[HD:128, :])
                    for s in range(4):  # V for four 128-token chunks
                        tc4 = 4 * n + s
                        pv = qq.tile([128, 512], F32, name=f"pv{n}_{s}", tag="mm")
                        for k in range(4):
                            nc.tensor.matmul(pv[:, :],
                                             xs[k][:, 128 * s:128 * (s + 1)],
                                             wvs[k][:, :],
                                             start=(k == 0), stop=(k == 3))
                        dst = vt[tc4].rearrange("p (h d) -> p h d", h=8)[:, :, 0:64]
                        src = pv.rearrange("p (h d) -> p h d", h=8)
                        if s % 2 == 0:
                            nc.vector.tensor_copy(dst, src)
                        else:
                            nc.scalar.copy(dst, src)

                # Q over own NQ tokens
                for n in range(NQ // 512):
                    xs = [wp.tile([128, 512], BF16, name=f"xq{n}_{k}",
                                  tag=f"xs{k}", bufs=2) for k in range(4)]
                    for k in range(4):
                        nc.sync.dma_start(
                            out=xs[k],
                            in_=xTq[128 * k:128 * (k + 1), 512 * n:512 * (n + 1)])
                    for p in range(4):
                        ps = qq.tile([128, 512], F32, name=f"pq{n}_{p}", tag="mm")
                        for k in range(4):
                            nc.tensor.matmul(ps[:, :],
                                             wqs[k][:, 128 * p:128 * (p + 1)],
                                             xs[k][:, :],
                                             start=(k == 0), stop=(k == 3))
                        if p % 2 == 0:
                            nc.scalar.copy(qa[2 * p][0:HD, 512 * n:512 * (n + 1)],
                                           ps[0:HD, :])
                            nc.scalar.copy(
                                qa[2 * p + 1][0:HD, 512 * n:512 * (n + 1)],
                                ps[HD:128, :])
                        else:
                            nc.vector.tensor_copy(
                                qa[2 * p][0:HD, 512 * n:512 * (n + 1)],
                                ps[0:HD, :])
                            nc.vector.tensor_copy(
                                qa[2 * p + 1][0:HD, 512 * n:512 * (n + 1)],
                                ps[HD:128, :])

            # ---- phase 2: attention ----
            for b in range(NBLK):
                c0, c1 = _block_chunks(b)
                nch = c1 - c0
                ngr = -(-nch // 4)
                for quad in range(2):
                    av = qq.tile([65, 512], F32, name=f"av{b}_{quad}", tag="av")
                    for hh in range(4):
                        h = 4 * quad + hh
                        exs = []
                        for g in range(ngr):
                            glo, ghi = 4 * g, min(4 * g + 4, nch)
                            sc = qq.tile([128, 512], F32,
                                         name=f"sc{b}_{h}_{g}", tag="sc", bufs=3)
                            for ci in range(glo, ghi):
                                j = ci - glo
                                nc.tensor.matmul(
                                    sc[:, 128 * j:128 * j + 128],
                                    ka[h][:, 128 * (c0 + ci):128 * (c0 + ci) + 128],
                                    qa[h][:, 128 * b:128 * (b + 1)],
                                    start=True, stop=True)
                            ex = sp.tile([128, 512], BF16,
                                         name=f"ex{b}_{h}_{g}", tag="ex", bufs=4)
                            nc.scalar.activation(
                                ex[:, 0:128 * (ghi - glo)],
                                sc[:, 0:128 * (ghi - glo)],
                                mybir.ActivationFunctionType.Exp)
                            exs.append(ex)
                        for ci in range(nch):
                            nc.tensor.matmul(
                                av[:, 128 * hh:128 * hh + 128],
                                vt[c0 + ci][:, 65 * h:65 * h + 65],
                                exs[ci // 4][:, 128 * (ci % 4):128 * (ci % 4) + 128],
                                start=(ci == 0), stop=(ci == nch - 1))
                    avs = sp.tile([65, 512], F32, name=f"avs{b}_{quad}", tag="avs")
                    nc.scalar.copy(avs[:, :], av[:, :])
                    rec = sp.tile([1, 512], F32R, name=f"rec{b}_{quad}", tag="rec",
                                  bufs=1)
                    with nc.allow_low_precision(reason="f32r recip for bcast"):
                        nc.vector.reciprocal(rec[:, :], avs[64:65, :])
                    bc = qq.tile([64, 512], F32, name=f"bc{b}_{quad}", tag="bc",
                                 bufs=1)
                    nc.tensor.matmul(bc[:, :], ones[:, :], rec[:, :],
                                     start=True, stop=True)
                    for hh in range(4):
                        h = 4 * quad + hh
                        nc.vector.tensor_mul(
                            ot[h // 2][64 * (h % 2):64 * (h % 2) + 64,
                                       128 * b:128 * (b + 1)],
                            avs[0:HD, 128 * hh:128 * hh + 128],
                            bc[:, 128 * hh:128 * hh + 128])

            # ---- phase 3: output projection ----
            for tch in range(NQ // 128):
                po = qq.tile([128, 512], F32, name=f"po{tch}", tag="mm")
                for p in range(4):
                    nc.tensor.matmul(po[:, :],
                                     ot[p][:, 128 * tch:128 * (tch + 1)],
                                     wos[p][:, :], start=(p == 0), stop=(p == 3))
                ob = sp.tile([128, 512], F32, name=f"ob{tch}", tag="ob")
                if tch % 2 == 0:
                    nc.vector.tensor_copy(ob[:, :], po[:, :])
                else:
                    nc.scalar.copy(ob[:, :], po[:, :])
                nc.sync.dma_start(out=out[128 * tch:128 * (tch + 1), :], in_=ob)

    nc.compile()
    return nc


def _host_inputs(hidden_states, w_q, w_k, w_v, w_o):
    bf = ml_dtypes.bfloat16
    xg = np.asarray(hidden_states, np.float32).reshape(H, W, T, C)
    xp = np.pad(xg, ((3, 3), (0, 0), (0, 0), (0, 0)))  # [38, W, T, C]

    # F: one-hot key position features, value NEG
    kk = np.arange(NKV)
    kw, khl, kt = kk // WCOL, (kk // T) % KVH, kk % T
    Fm = np.zeros((KPAD - HD, NKV), np.float32)
    Fm[kt, kk] = NEG
    Fm[R_T + khl, kk] = NEG
    Fm[R_T + R_H + kw, kk] = NEG
    Fm = Fm.astype(bf)

    qq_ = np.arange(NQ)
    qw, qj, qt = qq_ // (RH * T), (qq_ // T) % RH, qq_ % T
    ts = np.clip(qt - 3, 0, T - KT)
    wss = np.clip(qw - 3, 0, W - KS)

    wqb = np.asarray(w_q, np.float32).astype(bf)
    wkb = np.asarray(w_k, np.float32).astype(bf)
    wvb = np.asarray(w_v, np.float32).astype(bf)
    wob = np.asarray(w_o, np.float32).astype(bf)

    ins = []
    for i in range(N_CORES):
        # kv slice: global rows 4i-3 .. 4i+7 == padded rows 4i .. 4i+10
        xs = xp[4 * i:4 * i + KVH]                      # [10, W, T, C]
        xT = np.ascontiguousarray(
            xs.transpose(3, 1, 0, 2).reshape(C, NKV)).astype(bf)
        xTq = np.ascontiguousarray(
            xg[4 * i:4 * i + RH].transpose(3, 1, 0, 2).reshape(C, NQ)).astype(bf)

        Gm = np.zeros((KPAD - HD, NQ), np.float32)
        it = np.arange(R_T)[:, None]
        Gm[0:R_T] = ~((it >= ts[None, :]) & (it < ts[None, :] + KT))
        hglob = 4 * i + qj
        hs_loc = np.clip(hglob - 3, 0, H - KS) - (4 * i - 3)
        ih = np.arange(R_H)[:, None]
        Gm[R_T:R_T + R_H] = ~((ih >= hs_loc[None, :]) &
                              (ih < hs_loc[None, :] + KS))
        iw = np.arange(R_W)[:, None]
        Gm[R_T + R_H:NAUG] = ~((iw >= wss[None, :]) & (iw < wss[None, :] + KS))
        Gm = Gm.astype(bf)

        ins.append({
            "xT": xT, "xTq": xTq,
            "wq": wqb, "wk": wkb, "wv": wvb, "wo": wob,
            "Fm": Fm, "Gm": Gm,
            "one64": np.ones((1, HD), np.float32),
        })
    return ins


_NC_CACHE = None


def kernel(hidden_states, w_q, w_k, w_v, w_o, b_o):
    global _NC_CACHE
    if _NC_CACHE is None:
        _NC_CACHE = build_nc()
    nc = _NC_CACHE
    ins = _host_inputs(hidden_states, w_q, w_k, w_v, w_o)
    res = run_bass_kernel_spmd(nc, ins, core_ids=list(range(N_CORES)))

    full = np.empty((H, W, T, C), np.float32)
    for i in range(N_CORES):
        o = np.asarray(res.results[i]["out"]).reshape(W, RH, T, C)
        full[4 * i:4 * i + RH] = o.transpose(1, 0, 2, 3)
    full = full.reshape(H * W, T, C) + np.asarray(b_o, np.float32)
    return full

